# revision 16
# baseline (speedup 1.0000x reference)
"""Trainium2 Bass kernel for nn_Decoder_68152541053662.

2-layer GAT (heads=1, self-loops) + sigmoid inner-product decoder.
  N=12000 nodes, E=384000 edges (+N self loops), feats 40 -> 50 -> 40,
  output sigmoid(z @ z.T)  [12000, 12000] (bf16 on device, f32 on host).

Sharding: nodes row-partitioned across 8 cores (1500 dst rows each).
Each core aggregates only its own dst rows; full feature tables are
rebuilt on every core between layers with AllGather collectives.

Edge phase per layer (per core):
  - host packs incoming edges of each local dst into an ELL table
    (K=64 slots/dst, padded with a pointer to a dedicated pad row).
  - gathered rows come from an "augmented" bf16 feature table in DRAM:
    row = [h (F cols) | 1.0 | hi(a_src.h) | lo(a_src.h) | pad] (128 bf16
    = 256 B), fetched with dma_gather in 1024-idx chunks.  The a_src.h
    scalar is stored as a bf16 hi/lo pair: plain bf16 would quantize the
    softmax logit by ~0.03 (3% weight error); hi+lo reconstruction in
    f32 keeps it exact to ~1e-4.
  - attention weights w = exp(leaky_relu(hi+lo+ad[dst])) are computed in
    f32 on DVE/ACT, written as bf16 into a mostly-zero "selection
    matrix" M with a fixed strided pattern; aggregation is a chain of K
    accumulating bf16 matmuls with the GATHERED tile as the stationary
    operand:  aggT[f, dst] += T[:, t, 0:F+1].T @ M[:, t, :]
    which yields the TRANSPOSED aggregate (plus the softmax denominator
    in row F) and avoids both the 128-col LDWEIGHTS bottleneck of the
    M-stationary order and all downstream PE transposes.
  - epilogue: 1/denom broadcast via a rank-1 matmul, z = relu(agg*rec+b)
    produced directly in [F, 128] layout.
Decoder: S = sigmoid(z_own @ z_full.T), f32r matmuls in 512-col PSUM
slices, sigmoid applied over 2048-col PSUM spans (amortizes the ~352
cycle ACT instruction overhead), bf16 row strip written to DRAM.
"""

import numpy as np

try:
    import concourse.bass as bass
except ImportError:  # pragma: no cover
    import sys

    sys.path.insert(0, "/opt/trn_rl_repo")
    import concourse.bass as bass

import concourse.bacc as bacc
import concourse.tile as tile
from concourse import mybir
from concourse.bass_utils import run_bass_kernel_spmd

AF = mybir.ActivationFunctionType
ALU = mybir.AluOpType
F32 = mybir.dt.float32
F32R = mybir.dt.float32r
BF16 = mybir.dt.bfloat16
I16 = mybir.dt.int16

FULL_CFG = dict(N=12000, P=8, FIN=40, F1=50, F2=40, K=64, NEG=0.2, GCH=1024)
KTIERS = (32, 64, 128)  # per-block ELL widths (dividing 128 keeps the
                        # M-scatter bands 32-partition aligned)
DEBUG_EDGE_LVL = 3  # <3 truncates the edge phase (perf bisection only)
PAD_AS = -100.0  # "as" value of the pad row -> w ~ exp(-100) ~ 0


def derive(cfg):
    d = dict(cfg)
    d["NLOC"] = cfg["N"] // cfg["P"]
    d["NB"] = (d["NLOC"] + 127) // 128
    d["NLOCP"] = d["NB"] * 128
    d["NR"] = cfg["N"] + 1  # aug table rows (+1 pad row)
    # per-block ELL widths; uniform K until make_inputs computes the
    # degree-sorted schedule from the actual graph
    d.setdefault("KS", tuple([cfg["K"]] * d["NB"]))
    # decoder column chunking: 512-col matmul slices, sigmoid over DCW-col
    # PSUM spans
    d["DCW"] = 2048
    ch = []
    rem = cfg["N"]
    while rem > 0:
        w = min(d["DCW"], rem)
        # avoid a tiny trailing sigmoid: merge remainder < 512 into last
        if 0 < rem - w < 512:
            w = rem - 512
        ch.append(w)
        rem -= w
    d["DCHUNKS"] = ch
    assert cfg["N"] % cfg["P"] == 0 and 128 % cfg["K"] == 0
    return d


# --------------------------------------------------------------------------
# host-side preprocessing
# --------------------------------------------------------------------------


def _wrap16(flat):
    """Pack a flat int index list into dma_gather's replicated 16-wrap."""
    wrapped = np.zeros((128, len(flat) // 16), dtype=np.int16)
    cols = flat.reshape(-1, 16).T.astype(np.int16)  # [16, J//16]
    for g in range(8):
        wrapped[16 * g : 16 * g + 16, :] = cols
    return wrapped


def graph_plan(edge_index, cfg):
    """Degree-sorted per-core permutations + per-block K schedule.

    Each core sorts its 1500 nodes by in-degree (desc).  Block i's ELL
    width KS[i] is the smallest tier covering the max degree of block i
    across ALL cores (the SPMD program is shared).  Returns
    (perms, KS, deg): perms[core][pos] = original local index.
    """
    c = cfg
    N, P = c["N"], c["P"]
    nloc, nb = c["NLOC"], c["NB"]
    dst = np.asarray(edge_index[1], dtype=np.int64)
    deg = np.bincount(
        np.concatenate([dst, np.arange(N, dtype=np.int64)]), minlength=N
    )
    perms = [
        np.argsort(-deg[core * nloc : (core + 1) * nloc], kind="stable")
        for core in range(P)
    ]
    KS = []
    for b in range(nb):
        need = 0
        for core in range(P):
            idxs = perms[core][b * 128 : (b + 1) * 128]
            if len(idxs):
                need = max(need, int(deg[core * nloc + idxs].max()))
        KS.append(next(t for t in KTIERS if t >= need))
    return perms, KS, deg


def build_ell(edge_index, cfg, perms):
    """Per-core ELL + un-permute tables in the 16-wrapped int16 layout.

    Slot order within block b (width Kb): j = D*Kb + k, D = position of
    the dst within the block (degree-sorted order).  Gather indices point
    at haug rows, which live in PERMUTED per-core order:
    row(v) = core(v)*nloc + position of v in core(v)'s sort.
    """
    c = cfg
    N, P = c["N"], c["P"]
    nloc, nb, nlocp = c["NLOC"], c["NB"], c["NLOCP"]
    KS = c["KS"]
    src = np.asarray(edge_index[0], dtype=np.int64)
    dst = np.asarray(edge_index[1], dtype=np.int64)
    loops = np.arange(N, dtype=np.int64)
    src = np.concatenate([src, loops])
    dst = np.concatenate([dst, loops])

    order = np.argsort(dst, kind="stable")
    src, dst = src[order], dst[order]
    deg = np.bincount(dst, minlength=N)
    KMAX = max(KS)
    assert deg.max() <= KMAX, f"max degree {deg.max()} > {KMAX}"
    starts = np.concatenate([[0], np.cumsum(deg)])

    slots = np.full((N, KMAX), N, dtype=np.int64)
    pos = np.arange(len(dst)) - starts[dst]
    slots[dst, pos] = src

    # haug row of node v (permuted storage order); pad node N -> row N
    row_of = np.empty(N + 1, dtype=np.int64)
    row_of[N] = N
    invs = []
    for core in range(P):
        inv = np.empty(nloc, dtype=np.int64)
        inv[perms[core]] = np.arange(nloc)
        invs.append(inv)
        row_of[core * nloc : (core + 1) * nloc] = core * nloc + inv

    tabs, unps = [], []
    for core in range(P):
        parts = []
        for b in range(nb):
            kb = KS[b]
            blk = np.full((128, kb), N, dtype=np.int64)
            idxs = perms[core][b * 128 : (b + 1) * 128]
            if len(idxs):
                blk[: len(idxs)] = slots[core * nloc + idxs, :kb]
            parts.append(blk.reshape(-1))
        flat = row_of[np.concatenate(parts)]
        tabs.append(_wrap16(flat))
        unp = np.zeros(nlocp, dtype=np.int64)
        unp[:nloc] = invs[core]  # gathered row j = z of original local j
        unps.append(_wrap16(unp))
    return tabs, unps


def make_inputs(x, edge_index, W1, a_src1, a_dst1, b1, W2, a_src2, a_dst2, b2, cfg):
    c = cfg
    N, P, FIN, F1, F2 = c["N"], c["P"], c["FIN"], c["F1"], c["F2"]
    nloc, nlocp = c["NLOC"], c["NLOCP"]
    x = np.asarray(x, dtype=np.float32)
    perms, KS, _deg = graph_plan(edge_index, c)
    c["KS"] = tuple(KS)
    cfg["KS"] = tuple(KS)
    ell, unps = build_ell(edge_index, c, perms)

    def rep(v, f):
        r = np.zeros((128, f), dtype=np.float32)
        r[:] = np.asarray(v, dtype=np.float32)[None, :]
        return r

    def col(v):
        r = np.zeros((128, 1), dtype=np.float32)
        r[: len(v), 0] = np.asarray(v, dtype=np.float32)
        return r

    bf = np.dtype("bfloat16") if hasattr(np, "bfloat16") else None
    if bf is None:
        import ml_dtypes

        bf = np.dtype(ml_dtypes.bfloat16)

    # aug row layout: [h (fin) | as_hi@fin | as_lo@fin+1 | ... | 1.0@64]
    # (ones at col 64 so the denominator lands on a 32-aligned PSUM
    # partition row of the transposed aggregate)
    pads = np.zeros((2, 128), dtype=bf)
    pads[0, F1] = PAD_AS
    pads[1, F2] = PAD_AS
    pads[0, 64] = 1.0
    pads[1, 64] = 1.0

    common = {
        "w1": np.asarray(W1, dtype=np.float32),
        "w2": np.asarray(W2, dtype=np.float32),
        "asrc1r": rep(a_src1, F1),
        "adst1r": rep(a_dst1, F1),
        "asrc2r": rep(a_src2, F2),
        "adst2r": rep(a_dst2, F2),
        "b1t": col(b1),
        "b2t": col(b2),
        "pads": pads,
        "eye": np.eye(128, dtype=np.float32),
    }
    maps = []
    for core in range(P):
        xt = np.zeros((FIN, nlocp), dtype=np.float32)
        xt[:, :nloc] = x[core * nloc : (core + 1) * nloc][perms[core]].T
        m = dict(common)
        m["xt"] = xt
        m["elli"] = ell[core]
        m["unp"] = unps[core]
        maps.append(m)
    return maps


def bands(K):
    """Decompose the slot map (p, t) -> (D, k) into strided partition bands.

    Slot J = 128*t + p, D = J//K, k = J%K, t = T_p*s + r.  For each band
    (r, p0, p1, c): D = c + adv*s.  For K in {32, 64, 128} the p0 cuts are
    32-aligned, satisfying the engines' partition-base granularity.
    """
    from math import lcm

    T_p = lcm(K, 128) // 128
    adv = lcm(K, 128) // K
    out = []
    for r in range(T_p):
        J0 = 128 * r
        cuts = [0] + [p for p in range(1, 128) if (J0 + p) % K == 0] + [128]
        for p0, p1 in zip(cuts[:-1], cuts[1:]):
            out.append((r, p0, p1, (J0 + p0) // K))
    return T_p, adv, out


# --------------------------------------------------------------------------
# device program
# --------------------------------------------------------------------------


def build_program(cfg, stop_after=None, repeat=1, sim_cc=False):
    c = cfg
    N, P, FIN, F1, F2, K, NEG = (
        c["N"], c["P"], c["FIN"], c["F1"], c["F2"], c["K"], c["NEG"],
    )
    nloc, nb, nlocp, NR = c["NLOC"], c["NB"], c["NLOCP"], c["NR"]
    KS = list(c["KS"])
    KMAX = max(KS)
    ICOFF = [0]
    for kb in KS:
        ICOFF.append(ICOFF[-1] + 128 * kb // 16)  # idx cols are 16-wrapped
    GCH = c["GCH"]
    DCHUNKS = c["DCHUNKS"]
    tail = nloc - 128 * (nb - 1)
    groups = [list(range(P))]

    nc = bacc.Bacc(
        "TRN2",
        target_bir_lowering=False,
        debug=False,
        num_devices=P,
        dynamic_dma_scratch_size=32768,
    )

    # I/O
    xt_d = nc.dram_tensor("xt", [FIN, nlocp], F32, kind="ExternalInput")
    w1_d = nc.dram_tensor("w1", [FIN, F1], F32, kind="ExternalInput")
    w2_d = nc.dram_tensor("w2", [F1, F2], F32, kind="ExternalInput")
    asrc1r_d = nc.dram_tensor("asrc1r", [128, F1], F32, kind="ExternalInput")
    adst1r_d = nc.dram_tensor("adst1r", [128, F1], F32, kind="ExternalInput")
    asrc2r_d = nc.dram_tensor("asrc2r", [128, F2], F32, kind="ExternalInput")
    adst2r_d = nc.dram_tensor("adst2r", [128, F2], F32, kind="ExternalInput")
    b1t_d = nc.dram_tensor("b1t", [128, 1], F32, kind="ExternalInput")
    b2t_d = nc.dram_tensor("b2t", [128, 1], F32, kind="ExternalInput")
    pads_d = nc.dram_tensor("pads", [2, 128], BF16, kind="ExternalInput")
    elli_d = nc.dram_tensor("elli", [128, ICOFF[-1]], I16, kind="ExternalInput")
    unp_d = nc.dram_tensor("unp", [128, nlocp // 16], I16, kind="ExternalInput")
    eye_d = nc.dram_tensor("eye", [128, 128], F32, kind="ExternalInput")
    out_d = nc.dram_tensor("out", [nloc, N], BF16, kind="ExternalOutput")

    # internal DRAM
    haug1 = nc.dram_tensor("haug1", [NR, 128], BF16, addr_space="Shared")
    haug2 = nc.dram_tensor("haug2", [NR, 128], BF16, addr_space="Shared")
    ccin1 = nc.dram_tensor("ccin1", [nloc, 128], BF16)
    ccout1 = nc.dram_tensor("ccout1", [N, 128], BF16, addr_space="Shared")
    ccin2 = nc.dram_tensor("ccin2", [nloc, 128], BF16)
    ccout2 = nc.dram_tensor("ccout2", [N, 128], BF16, addr_space="Shared")
    adt1 = nc.dram_tensor("adt1", [nb, 128], F32)
    adt2 = nc.dram_tensor("adt2", [nb, 128], F32)
    zperm = nc.dram_tensor("zperm", [nlocp, 64], F32)
    ztin = nc.dram_tensor("ztin", [F2, nloc], F32)
    ztcc = nc.dram_tensor("ztcc", [P * F2, nloc], F32, addr_space="Shared")

    with tile.TileContext(nc) as tc:
      with tc.tile_pool(name="persist", bufs=1) as ppool:
        zt_own = ppool.tile([F2, nlocp], F32R)
        def _pipeline():
            with (
                tc.tile_pool(name="const", bufs=1) as cpool,
                tc.tile_pool(name="strips", bufs=1) as spool,
                tc.tile_pool(name="gat_small", bufs=3) as gpool,
                tc.tile_pool(name="gather", bufs=2) as tpool,
                tc.tile_pool(name="psum_small", bufs=2, space="PSUM") as pps,
                tc.tile_pool(name="psum_agg", bufs=3, space="PSUM") as pagg,
                tc.tile_pool(name="psum_bc", bufs=2, space="PSUM") as pbc,
            ):
                # ---- constant loads -------------------------------------------------
                xt_sb = cpool.tile([FIN, nlocp], F32)
                nc.sync.dma_start(out=xt_sb[:, :], in_=xt_d[:, :])
                w1_sb = cpool.tile([FIN, F1], F32)
                nc.sync.dma_start(out=w1_sb[:, :], in_=w1_d[:, :])
                w2_sb = cpool.tile([F1, F2], F32)
                nc.sync.dma_start(out=w2_sb[:, :], in_=w2_d[:, :])
                asrc1_sb = cpool.tile([128, F1], F32)
                nc.sync.dma_start(out=asrc1_sb[:, :], in_=asrc1r_d[:, :])
                adst1_sb = cpool.tile([128, F1], F32)
                nc.sync.dma_start(out=adst1_sb[:, :], in_=adst1r_d[:, :])
                asrc2_sb = cpool.tile([128, F2], F32)
                nc.sync.dma_start(out=asrc2_sb[:, :], in_=asrc2r_d[:, :])
                adst2_sb = cpool.tile([128, F2], F32)
                nc.sync.dma_start(out=adst2_sb[:, :], in_=adst2r_d[:, :])
                b1t_sb = cpool.tile([128, 1], F32)
                nc.sync.dma_start(out=b1t_sb[:, :], in_=b1t_d[:, :])
                b2t_sb = cpool.tile([128, 1], F32)
                nc.sync.dma_start(out=b2t_sb[:, :], in_=b2t_d[:, :])
                elli_sb = cpool.tile([128, ICOFF[-1]], I16)
                nc.sync.dma_start(out=elli_sb[:, :], in_=elli_d[:, :])
                unp_sb = cpool.tile([128, nlocp // 16], I16)
                nc.sync.dma_start(out=unp_sb[:, :], in_=unp_d[:, :])
                eye_sb = cpool.tile([128, 128], F32)
                nc.sync.dma_start(out=eye_sb[:, :], in_=eye_d[:, :])
                ones1_sb = cpool.tile([1, 128], F32)
                nc.vector.memset(ones1_sb[:, :], 1.0)
                onesb_sb = cpool.tile([1, 64], BF16)
                nc.vector.memset(onesb_sb[:, :], 1.0)

                # selection matrix M: [128, K, 128] bf16, zeroed once; the
                # non-zero pattern (G strided diagonals) is identical for
                # every block, so reused buffers never need re-zeroing.
                m_sbs = []
                m_lastk = [KMAX, KMAX]
                for _mi in range(2):
                    m_i = cpool.tile([128, KMAX, 128], BF16, name=f"m{_mi}", tag=f"m{_mi}")
                    nc.vector.memset(m_i[:, :, :], 0.0)
                    m_sbs.append(m_i)

                strip = spool.tile([128, nb, 128], BF16)
                nc.vector.memset(strip[:, :, :], 0.0)
                zstrip = spool.tile([128, nb, 64], F32)
                nc.vector.memset(zstrip[:, :, :], 0.0)
                asv = spool.tile([128, nb], F32)
                adv1 = spool.tile([128, nb], F32)
                adv2 = spool.tile([128, nb], F32)
                adrep1 = spool.tile([128, nlocp], F32)
                adrep2 = spool.tile([128, nlocp], F32)

                # ---- helpers --------------------------------------------------------
                def hi_lo_to_strip(col0):
                    """Store asv [128, nb] f32 into strip cols col0 (bf16 hi)
                    and col0+1 (bf16 lo = asv - f32(hi))."""
                    hi_bf = gpool.tile([128, nb], BF16, tag="hibf")
                    nc.vector.tensor_copy(hi_bf[:, :], asv[:, :])
                    nc.vector.tensor_copy(strip[:, :, col0], hi_bf[:, :])
                    hi_f = gpool.tile([128, nb], F32, tag="hif")
                    nc.vector.tensor_copy(hi_f[:, :], hi_bf[:, :])
                    nc.vector.tensor_sub(hi_f[:, :], asv[:, :], hi_f[:, :])
                    nc.vector.tensor_copy(strip[:, :, col0 + 1], hi_f[:, :])

                def adrep_roundtrip(adv, adt_dram, adrep):
                    """adv [128, nb] (val for dst 128*b+p) -> adrep [128, nlocp]
                    (row-replicated) via DRAM roundtrip + ones-matmul broadcast."""
                    nc.sync.dma_start(out=adt_dram.ap().rearrange("b p -> p b"), in_=adv[:, :])
                    adrow = gpool.tile([1, nlocp], F32, tag="adrow")
                    nc.sync.dma_start(
                        out=adrow[:, :], in_=adt_dram.ap().flatten().unsqueeze(0)
                    )
                    for j0 in range(0, nlocp, 512):
                        w = min(512, nlocp - j0)
                        pt = pps.tile([128, 512], F32, tag="ps", name="ps")
                        nc.tensor.matmul(
                            pt[:, :w], ones1_sb[:, :], adrow[:, j0 : j0 + w],
                            start=True, stop=True,
                        )
                        nc.vector.tensor_copy(adrep[:, j0 : j0 + w], pt[:, :w])

                def strip_out(ccin):
                    if nb > 1:
                        nc.sync.dma_start(
                            out=ccin.ap()[0 : 128 * (nb - 1), :].rearrange(
                                "(b p) f -> p b f", p=128
                            ),
                            in_=strip[:, 0 : nb - 1, :],
                        )
                    nc.sync.dma_start(
                        out=ccin.ap()[128 * (nb - 1) : nloc, :],
                        in_=strip[0:tail, nb - 1, :],
                    )

                def allgather(ccin, ccout, haug, pad_idx):
                    # gather straight into haug[0:N] (no staging copy)
                    if sim_cc:
                        # TimelineSim proxy: approximate the AllGather's local
                        # receive traffic with P DRAM->DRAM copies.
                        for r in range(P):
                            nc.sync.dma_start(
                                out=haug.ap()[r * nloc : (r + 1) * nloc, :],
                                in_=ccin.ap()[:, :],
                            )
                    else:
                        nc.gpsimd.collective_compute(
                            "AllGather",
                            ALU.bypass,
                            replica_groups=groups,
                            ins=[ccin.ap().opt()],
                            outs=[haug.ap()[0:N, :].opt()],
                        )
                    nc.sync.dma_start(
                        out=haug.ap()[N : N + 1, :], in_=pads_d.ap()[pad_idx : pad_idx + 1, :]
                    )

                stopped = False

                def _dummy_out():
                    dz = gpool.tile([128, 512], BF16, tag="dz")
                    nc.vector.memset(dz[:, :], 0.0)
                    nc.sync.dma_start(out=out_d.ap()[0:128, 0:512], in_=dz[:, :])

                # ---- phase B: layer-1 linear on own nodes --------------------------
                scr = gpool.tile([128, F1], F32, tag="scr")
                for b in range(nb):
                    ph = pps.tile([128, 512], F32, tag="ps", name="ps")[:, 0:F1]
                    nc.tensor.matmul(
                        ph[:, :], xt_sb[:, 128 * b : 128 * (b + 1)], w1_sb[:, :],
                        start=True, stop=True,
                    )
                    nc.vector.tensor_copy(strip[:, b, 0:F1], ph[:, :])
                    nc.vector.memset(strip[:, b, 64:65], 1.0)
                    nc.vector.tensor_mul(scr[:, :], ph[:, :], asrc1_sb[:, :])
                    nc.vector.reduce_sum(
                        asv[:, b : b + 1], scr[:, :], axis=mybir.AxisListType.X
                    )
                    nc.vector.tensor_mul(scr[:, :], ph[:, :], adst1_sb[:, :])
                    nc.vector.reduce_sum(
                        adv1[:, b : b + 1], scr[:, :], axis=mybir.AxisListType.X
                    )
                hi_lo_to_strip(F1)
                if stop_after == "B0":
                    _dummy_out(); stopped = True
                if not stopped:
                    strip_out(ccin1)
                    allgather(ccin1, ccout1, haug1, 0)
                    adrep_roundtrip(adv1, adt1, adrep1)
                if stop_after == "B" and not stopped:
                    _dummy_out(); stopped = True

                # ---- edge layer ----------------------------------------------------
                EDGE_LVL = DEBUG_EDGE_LVL  # 3 = full edge phase (debug knob)

                def edge_layer(haug, adrep, fin, bt_sb, out_block, order):
                    """Aggregate one GAT layer for all own blocks.

                    haug rows (bf16): [h (fin) | 1 | as_hi | as_lo | junk]; for
                    each block produces zT = relu(aggT*rec + b) [fin, 128] and
                    calls out_block(b, zT).
                    """
                    scol = 64  # ones column -> denominator row of aggT
                    acol = fin  # as_hi / as_lo columns
                    rNc = 65  # matmul lhsT width: h cols .. ones col
                    for bi, b in enumerate(order):
                        Kb = KS[b]
                        T_p, adv, bnd = bands(Kb)
                        nS = Kb // T_p
                        T = tpool.tile([128, KMAX, 128], BF16, tag="T")
                        for q in range(128 * Kb // GCH):
                            nc.gpsimd.dma_gather(
                                T[:, q * (GCH // 128) : (q + 1) * (GCH // 128), :],
                                haug.ap()[:, :],
                                elli_sb[
                                    :,
                                    ICOFF[b] + q * (GCH // 16) : ICOFF[b] + (q + 1) * (GCH // 16),
                                ],
                                GCH,
                                GCH,
                                128,
                            )
                        if EDGE_LVL < 1:
                            continue
                        adT = gpool.tile([128, KMAX], F32, tag="adT")
                        for r, p0, p1, cc in bnd:
                            nc.vector.tensor_copy(
                                adT[p0:p1, r:Kb:T_p],
                                adrep[
                                    p0:p1,
                                    128 * b + cc : 128 * b + cc + adv * (nS - 1) + 1 : adv,
                                ],
                            )
                        e = gpool.tile([128, KMAX], F32, tag="e")
                        e2 = gpool.tile([128, KMAX], F32, tag="e2")
                        nc.vector.tensor_copy(e[:, 0:Kb], T[:, 0:Kb, acol])
                        nc.vector.tensor_copy(e2[:, 0:Kb], T[:, 0:Kb, acol + 1])
                        nc.vector.tensor_add(e[:, 0:Kb], e[:, 0:Kb], e2[:, 0:Kb])
                        nc.vector.tensor_add(e[:, 0:Kb], e[:, 0:Kb], adT[:, 0:Kb])
                        # leaky_relu(e) = max(e, NEG*e), then exp on ACT
                        nc.vector.scalar_tensor_tensor(
                            e[:, 0:Kb], e[:, 0:Kb], NEG, e[:, 0:Kb], ALU.mult, ALU.max
                        )
                        ew = gpool.tile([128, KMAX], BF16, tag="ew")
                        nc.scalar.activation(ew[:, 0:Kb], e[:, 0:Kb], AF.Exp)
                        # scatter w into the fixed M pattern for width Kb;
                        # a buffer last used with a larger K keeps stale
                        # non-zeros outside the new pattern -> re-zero it
                        m_sb = m_sbs[bi % 2]
                        if Kb != m_lastk[bi % 2]:
                            # a K change moves every diagonal: clear the whole
                            # region either pattern touches
                            nc.vector.memset(
                                m_sb[:, 0 : max(Kb, m_lastk[bi % 2]), :], 0.0
                            )
                        m_lastk[bi % 2] = Kb
                        mv = m_sb[:, :, :].rearrange("p a b -> p (a b)")
                        cstr = 128 * T_p + adv
                        for r, p0, p1, cc in bnd:
                            nc.vector.tensor_copy(
                                mv[
                                    p0:p1,
                                    128 * r + cc : 128 * r + cc + (nS - 1) * cstr + 1 : cstr,
                                ],
                                ew[p0:p1, r:Kb:T_p],
                            )
                        if EDGE_LVL < 2:
                            continue
                        agg = pagg.tile([128, 128], F32, tag="agg")
                        for t in range(Kb):
                            nc.tensor.matmul(
                                agg[0:rNc, :],
                                T[:, t, 0:rNc],
                                m_sb[:, t, :],
                                start=(t == 0),
                                stop=(t == Kb - 1),
                            )
                        if EDGE_LVL < 3:
                            continue
                        rec = gpool.tile([1, 128], F32, tag="rec")
                        nc.vector.reciprocal(rec[:, :], agg[scol : scol + 1, :])
                        recb = gpool.tile([1, 128], BF16, tag="recb")
                        nc.vector.tensor_copy(recb[:, :], rec[:, :])
                        bc = pbc.tile([128, 128], F32, tag="bc")
                        nc.tensor.matmul(
                            bc[0:fin, :], onesb_sb[:, 0:fin], recb[:, :],
                            start=True, stop=True,
                        )
                        # DVE can't read two PSUM operands in one op: stage
                        # the broadcast in SBUF first.
                        bcs = gpool.tile([128, 128], F32, tag="bcs")
                        nc.vector.tensor_copy(bcs[0:fin, :], bc[0:fin, :])
                        z = gpool.tile([128, 128], F32, tag="z")
                        nc.vector.tensor_mul(z[0:fin, :], agg[0:fin, :], bcs[0:fin, :])
                        nc.vector.tensor_scalar(
                            z[0:fin, :], z[0:fin, :], bt_sb[0:fin, 0:1], 0.0,
                            ALU.add, ALU.max,
                        )
                        out_block(b, z)

                # ---- layer-1 consumer: h2 = z1 @ W2, rebuild strip -----------------
                def l1_out(b, z):
                    ph2 = pps.tile([128, 512], F32, tag="ps", name="ps")[:, 0:F2]
                    nc.tensor.matmul(
                        ph2[:, :], z[0:F1, :], w2_sb[:, :], start=True, stop=True
                    )
                    nc.vector.tensor_copy(strip[:, b, 0:F2], ph2[:, :])
                    nc.vector.memset(strip[:, b, 64:65], 1.0)
                    scr2 = gpool.tile([128, F2], F32, tag="scr2")
                    nc.vector.tensor_mul(scr2[:, :], ph2[:, :], asrc2_sb[:, :])
                    nc.vector.reduce_sum(
                        asv[:, b : b + 1], scr2[:, :], axis=mybir.AxisListType.X
                    )
                    nc.vector.tensor_mul(scr2[:, :], ph2[:, :], adst2_sb[:, :])
                    nc.vector.reduce_sum(
                        adv2[:, b : b + 1], scr2[:, :], axis=mybir.AxisListType.X
                    )

                if not stopped:
                    edge_layer(haug1, adrep1, F1, b1t_sb, l1_out, list(range(nb)))
                    hi_lo_to_strip(F2)
                if stop_after == "C" and not stopped:
                    _dummy_out(); stopped = True
                if not stopped:
                    strip_out(ccin2)
                    allgather(ccin2, ccout2, haug2, 1)
                    adrep_roundtrip(adv2, adt2, adrep2)

                # ---- layer-2 consumer: z rows (dst-major) into zstrip --------------
                def l2_out(b, z):
                    zp = pps.tile([128, 512], F32, tag="ps", name="ps")[:, 0:128]
                    nc.tensor.transpose(
                        zp[:, 0:F2], z[0:F2, :], eye_sb[0:F2, 0:F2]
                    )
                    nc.vector.tensor_copy(zstrip[:, b, 0:F2], zp[:, 0:F2])

                if not stopped:
                    # reverse order: layer 1 ends on the small-K tail, layer 2
                    # starts there -> only one K transition per layer
                    edge_layer(haug2, adrep2, F2, b2t_sb, l2_out, list(range(nb))[::-1])
                if stop_after == "D" and not stopped:
                    _dummy_out(); stopped = True

                if not stopped:
                    # un-permute: zstrip (sorted order) -> DRAM -> gather rows
                    # back in original order -> transpose into zt_own
                    nc.sync.dma_start(
                        out=zperm.ap().rearrange("(b p) f -> p b f", p=128),
                        in_=zstrip[:, :, :],
                    )
                    zrows = spool.tile([128, nb, 64], F32)
                    UGC = 512  # keep single-gather descriptor count small
                    for q in range(nlocp // UGC):
                        nc.gpsimd.dma_gather(
                            zrows[:, q * (UGC // 128) : (q + 1) * (UGC // 128), :],
                            zperm.ap()[:, :],
                            unp_sb[:, q * (UGC // 16) : (q + 1) * (UGC // 16)],
                            UGC,
                            UGC,
                            64,
                        )
                    for b in range(nb):
                        zp2 = pps.tile([128, 512], F32, tag="ps", name="ps")[:, 0:128]
                        nc.tensor.transpose(
                            zp2[0:F2, :], zrows[:, b, 0:F2], eye_sb[:, :]
                        )
                        nc.vector.tensor_copy(
                            zt_own[:, 128 * b : 128 * (b + 1)], zp2[0:F2, :]
                        )
                    # share z (transposed) with all cores
                    nc.sync.dma_start(out=ztin.ap()[:, :], in_=zt_own[:, 0:nloc].bitcast(F32))
                    if sim_cc:
                        for r in range(P):
                            nc.sync.dma_start(
                                out=ztcc.ap()[r * F2 : (r + 1) * F2, :],
                                in_=ztin.ap()[:, :],
                            )
                    else:
                        nc.gpsimd.collective_compute(
                            "AllGather",
                            ALU.bypass,
                            replica_groups=groups,
                            ins=[ztin.ap().opt()],
                            outs=[ztcc.ap().opt()],
                        )
                else:
                    nc.vector.memset(zt_own[:, :], 0.0)
                    nc.sync.dma_start(out=ztcc.ap()[0:F2, :], in_=zt_own[:, 0:nloc].bitcast(F32))

            # ---- decoder (separate pool scope so GAT SBUF is reusable) ------------
            with (
                tc.tile_pool(name="dec", bufs=1) as dpool,
                tc.tile_pool(name="dec_rows", bufs=2) as rpool,
                tc.tile_pool(name="psum_dec", bufs=2, space="PSUM") as pdec,
            ):
                if stopped:
                    P_eff = 0
                    nb_eff = 0
                else:
                    P_eff = P
                    nb_eff = nb
                ztf = dpool.tile([F2, N], F32)
                for r in range(P_eff):
                    nc.sync.dma_start(
                        out=ztf[:, r * nloc : (r + 1) * nloc],
                        in_=ztcc.ap()[r * F2 : (r + 1) * F2, :],
                    )
                for b in range(nb_eff):
                    rows = 128 if b < nb - 1 else tail
                    srow = rpool.tile([128, N], BF16, tag="srow")
                    j0 = 0
                    for W in DCHUNKS:
                        pd = pdec.tile([128, c["DCW"]], F32, tag="pd")
                        for s0 in range(0, W, 512):
                            w = min(512, W - s0)
                            nc.tensor.matmul(
                                pd[:, s0 : s0 + w],
                                zt_own[:, 128 * b : 128 * (b + 1)],
                                ztf[:, j0 + s0 : j0 + s0 + w].bitcast(F32R),
                                start=True,
                                stop=True,
                            )
                        nc.scalar.activation(
                            srow[:, j0 : j0 + W], pd[:, 0:W], AF.Sigmoid
                        )
                        j0 += W
                    nc.sync.dma_start(
                        out=out_d.ap()[128 * b : 128 * b + rows, :], in_=srow[0:rows, :]
                    )

        for _rep in range(repeat):
            _pipeline()
            if stop_after is not None and repeat > 1:
                tc.strict_bb_all_engine_barrier()

    nc.compile()
    return nc


# --------------------------------------------------------------------------
# entry point
# --------------------------------------------------------------------------

_CACHE = {}
TRACE = False
LAST_RESULT = None


def kernel(x, edge_index, W1, a_src1, a_dst1, b1, W2, a_src2, a_dst2, b2, **_):
    base = dict(FULL_CFG)
    # ELL width: 64 covers the reference graph (max in-degree 55); fall back
    # to 128 for denser graphs.
    ei = np.asarray(edge_index)
    deg = np.bincount(
        np.concatenate([ei[1].astype(np.int64), np.arange(base["N"])]),
        minlength=base["N"],
    )
    if deg.max() > 64:
        base["K"] = 128
    cfg = derive(base)
    maps = make_inputs(
        x, edge_index, W1, a_src1, a_dst1, b1, W2, a_src2, a_dst2, b2, cfg
    )
    key = ("full", base["K"])
    if key not in _CACHE:
        _CACHE[key] = build_program(cfg)
    nc = _CACHE[key]
    global LAST_RESULT
    res = run_bass_kernel_spmd(nc, maps, list(range(cfg["P"])), trace=TRACE)
    LAST_RESULT = res
    out = np.concatenate(
        [np.asarray(res.results[i]["out"]) for i in range(cfg["P"])], axis=0
    )
    return out.astype(np.float32)


# revision 19
# speedup vs baseline: 2.2121x; 2.2121x over previous
"""Trainium2 Bass kernel for nn_Decoder_68152541053662.

2-layer GAT (heads=1, self-loops) + sigmoid inner-product decoder.
  N=12000 nodes, E=384000 edges (+N self loops), feats 40 -> 50 -> 40,
  output sigmoid(z @ z.T)  [12000, 12000] (bf16 on device, f32 on host).

Sharding: nodes row-partitioned across 8 cores (1500 dst rows each).
Each core aggregates only its own dst rows; full feature tables are
rebuilt on every core between layers with AllGather collectives.

Edge phase per layer (per core):
  - host packs incoming edges of each local dst into an ELL table
    (K=64 slots/dst, padded with a pointer to a dedicated pad row).
  - gathered rows come from an "augmented" bf16 feature table in DRAM:
    row = [h (F cols) | 1.0 | hi(a_src.h) | lo(a_src.h) | pad] (128 bf16
    = 256 B), fetched with dma_gather in 1024-idx chunks.  The a_src.h
    scalar is stored as a bf16 hi/lo pair: plain bf16 would quantize the
    softmax logit by ~0.03 (3% weight error); hi+lo reconstruction in
    f32 keeps it exact to ~1e-4.
  - attention weights w = exp(leaky_relu(hi+lo+ad[dst])) are computed in
    f32 on DVE/ACT, written as bf16 into a mostly-zero "selection
    matrix" M with a fixed strided pattern; aggregation is a chain of K
    accumulating bf16 matmuls with the GATHERED tile as the stationary
    operand:  aggT[f, dst] += T[:, t, 0:F+1].T @ M[:, t, :]
    which yields the TRANSPOSED aggregate (plus the softmax denominator
    in row F) and avoids both the 128-col LDWEIGHTS bottleneck of the
    M-stationary order and all downstream PE transposes.
  - epilogue: 1/denom broadcast via a rank-1 matmul, z = relu(agg*rec+b)
    produced directly in [F, 128] layout.
Decoder: S = sigmoid(z_own @ z_full.T), f32r matmuls in 512-col PSUM
slices, sigmoid applied over 2048-col PSUM spans (amortizes the ~352
cycle ACT instruction overhead), bf16 row strip written to DRAM.
"""

import numpy as np

try:
    import concourse.bass as bass
except ImportError:  # pragma: no cover
    import sys

    sys.path.insert(0, "/opt/trn_rl_repo")
    import concourse.bass as bass

import concourse.bacc as bacc
import concourse.tile as tile
from concourse import mybir
from concourse.bass_utils import run_bass_kernel_spmd

AF = mybir.ActivationFunctionType
ALU = mybir.AluOpType
F32 = mybir.dt.float32
F32R = mybir.dt.float32r
BF16 = mybir.dt.bfloat16
I16 = mybir.dt.int16

FULL_CFG = dict(N=12000, P=8, FIN=40, F1=50, F2=40, K=64, NEG=0.2, GCH=1024)
KTIERS = (32, 64, 128)  # per-block ELL widths (dividing 128 keeps the
                        # M-scatter bands 32-partition aligned)
DEBUG_EDGE_LVL = 3  # <3 truncates the edge phase (perf bisection only)
PAD_AS = -100.0  # "as" value of the pad row -> w ~ exp(-100) ~ 0


def derive(cfg):
    d = dict(cfg)
    d["NLOC"] = cfg["N"] // cfg["P"]
    d["NB"] = (d["NLOC"] + 127) // 128
    d["NLOCP"] = d["NB"] * 128
    d["NR"] = cfg["N"] + 1  # aug table rows (+1 pad row)
    # per-block ELL widths; uniform K until make_inputs computes the
    # degree-sorted schedule from the actual graph
    d.setdefault("KS", tuple([cfg["K"]] * d["NB"]))
    # decoder column chunking: 512-col matmul slices, sigmoid over DCW-col
    # PSUM spans
    d["DCW"] = 2048
    ch = []
    rem = cfg["N"]
    while rem > 0:
        w = min(d["DCW"], rem)
        # avoid a tiny trailing sigmoid: merge remainder < 512 into last
        if 0 < rem - w < 512:
            w = rem - 512
        ch.append(w)
        rem -= w
    d["DCHUNKS"] = ch
    assert cfg["N"] % cfg["P"] == 0 and 128 % cfg["K"] == 0
    return d


# --------------------------------------------------------------------------
# host-side preprocessing
# --------------------------------------------------------------------------


def _wrap16(flat):
    """Pack a flat int index list into dma_gather's replicated 16-wrap."""
    wrapped = np.zeros((128, len(flat) // 16), dtype=np.int16)
    cols = flat.reshape(-1, 16).T.astype(np.int16)  # [16, J//16]
    for g in range(8):
        wrapped[16 * g : 16 * g + 16, :] = cols
    return wrapped


def graph_plan(edge_index, cfg):
    """Degree-sorted per-core permutations + per-block K schedule.

    Each core sorts its 1500 nodes by in-degree (desc).  Block i's ELL
    width KS[i] is the smallest tier covering the max degree of block i
    across ALL cores (the SPMD program is shared).  Returns
    (perms, KS, deg): perms[core][pos] = original local index.
    """
    c = cfg
    N, P = c["N"], c["P"]
    nloc, nb = c["NLOC"], c["NB"]
    dst = np.asarray(edge_index[1], dtype=np.int64)
    deg = np.bincount(
        np.concatenate([dst, np.arange(N, dtype=np.int64)]), minlength=N
    )
    perms = [
        np.argsort(-deg[core * nloc : (core + 1) * nloc], kind="stable")
        for core in range(P)
    ]
    KS = []
    for b in range(nb):
        need = 0
        for core in range(P):
            idxs = perms[core][b * 128 : (b + 1) * 128]
            if len(idxs):
                need = max(need, int(deg[core * nloc + idxs].max()))
        KS.append(next(t for t in KTIERS if t >= need))
    return perms, KS, deg


def build_ell(edge_index, cfg, perms):
    """Per-core ELL + un-permute tables in the 16-wrapped int16 layout.

    Slot order within block b (width Kb): j = D*Kb + k, D = position of
    the dst within the block (degree-sorted order).  Gather indices point
    at haug rows, which live in PERMUTED per-core order:
    row(v) = core(v)*nloc + position of v in core(v)'s sort.
    """
    c = cfg
    N, P = c["N"], c["P"]
    nloc, nb, nlocp = c["NLOC"], c["NB"], c["NLOCP"]
    KS = c["KS"]
    src = np.asarray(edge_index[0], dtype=np.int64)
    dst = np.asarray(edge_index[1], dtype=np.int64)
    loops = np.arange(N, dtype=np.int64)
    src = np.concatenate([src, loops])
    dst = np.concatenate([dst, loops])

    order = np.argsort(dst, kind="stable")
    src, dst = src[order], dst[order]
    deg = np.bincount(dst, minlength=N)
    KMAX = max(KS)
    assert deg.max() <= KMAX, f"max degree {deg.max()} > {KMAX}"
    starts = np.concatenate([[0], np.cumsum(deg)])

    slots = np.full((N, KMAX), N, dtype=np.int64)
    pos = np.arange(len(dst)) - starts[dst]
    slots[dst, pos] = src

    # haug row of node v (permuted storage order); pad node N -> row N
    row_of = np.empty(N + 1, dtype=np.int64)
    row_of[N] = N
    invs = []
    for core in range(P):
        inv = np.empty(nloc, dtype=np.int64)
        inv[perms[core]] = np.arange(nloc)
        invs.append(inv)
        row_of[core * nloc : (core + 1) * nloc] = core * nloc + inv

    tabs, unps = [], []
    for core in range(P):
        parts = []
        for b in range(nb):
            kb = KS[b]
            blk = np.full((128, kb), N, dtype=np.int64)
            idxs = perms[core][b * 128 : (b + 1) * 128]
            if len(idxs):
                blk[: len(idxs)] = slots[core * nloc + idxs, :kb]
            parts.append(blk.reshape(-1))
        flat = row_of[np.concatenate(parts)]
        tabs.append(_wrap16(flat))
        unp = np.zeros(nlocp, dtype=np.int64)
        unp[:nloc] = invs[core]  # gathered row j = z of original local j
        unps.append(_wrap16(unp))
    return tabs, unps


def make_inputs(x, edge_index, W1, a_src1, a_dst1, b1, W2, a_src2, a_dst2, b2, cfg):
    c = cfg
    N, P, FIN, F1, F2 = c["N"], c["P"], c["FIN"], c["F1"], c["F2"]
    nloc, nlocp = c["NLOC"], c["NLOCP"]
    x = np.asarray(x, dtype=np.float32)
    perms, KS, _deg = graph_plan(edge_index, c)
    c["KS"] = tuple(KS)
    cfg["KS"] = tuple(KS)
    ell, unps = build_ell(edge_index, c, perms)

    def rep(v, f):
        r = np.zeros((128, f), dtype=np.float32)
        r[:] = np.asarray(v, dtype=np.float32)[None, :]
        return r

    def col(v):
        r = np.zeros((128, 1), dtype=np.float32)
        r[: len(v), 0] = np.asarray(v, dtype=np.float32)
        return r

    bf = np.dtype("bfloat16") if hasattr(np, "bfloat16") else None
    if bf is None:
        import ml_dtypes

        bf = np.dtype(ml_dtypes.bfloat16)

    # aug row layout: [h (fin) | as_hi@fin | as_lo@fin+1 | ... | 1.0@64]
    # (ones at col 64 so the denominator lands on a 32-aligned PSUM
    # partition row of the transposed aggregate)
    pads = np.zeros((2, 128), dtype=bf)
    pads[0, F1] = PAD_AS
    pads[1, F2] = PAD_AS
    pads[0, 64] = 1.0
    pads[1, 64] = 1.0

    common = {
        "w1": np.asarray(W1, dtype=np.float32),
        "w2": np.asarray(W2, dtype=np.float32),
        "asrc1r": rep(a_src1, F1),
        "adst1r": rep(a_dst1, F1),
        "asrc2r": rep(a_src2, F2),
        "adst2r": rep(a_dst2, F2),
        "b1t": col(b1),
        "b2t": col(b2),
        "pads": pads,
        "eye": np.eye(128, dtype=np.float32),
    }
    maps = []
    for core in range(P):
        xt = np.zeros((FIN, nlocp), dtype=np.float32)
        xt[:, :nloc] = x[core * nloc : (core + 1) * nloc][perms[core]].T
        m = dict(common)
        m["xt"] = xt
        m["elli"] = ell[core]
        m["unp"] = unps[core]
        maps.append(m)
    return maps


def bands(K):
    """Decompose the slot map (p, t) -> (D, k) into strided partition bands.

    Slot J = 128*t + p, D = J//K, k = J%K, t = T_p*s + r.  For each band
    (r, p0, p1, c): D = c + adv*s.  For K in {32, 64, 128} the p0 cuts are
    32-aligned, satisfying the engines' partition-base granularity.
    """
    from math import lcm

    T_p = lcm(K, 128) // 128
    adv = lcm(K, 128) // K
    out = []
    for r in range(T_p):
        J0 = 128 * r
        cuts = [0] + [p for p in range(1, 128) if (J0 + p) % K == 0] + [128]
        for p0, p1 in zip(cuts[:-1], cuts[1:]):
            out.append((r, p0, p1, (J0 + p0) // K))
    return T_p, adv, out


# --------------------------------------------------------------------------
# device program
# --------------------------------------------------------------------------


def build_program(cfg, stop_after=None, repeat=1, sim_cc=False):
    c = cfg
    N, P, FIN, F1, F2, K, NEG = (
        c["N"], c["P"], c["FIN"], c["F1"], c["F2"], c["K"], c["NEG"],
    )
    nloc, nb, nlocp, NR = c["NLOC"], c["NB"], c["NLOCP"], c["NR"]
    KS = list(c["KS"])
    KMAX = max(KS)
    ICOFF = [0]
    for kb in KS:
        ICOFF.append(ICOFF[-1] + 128 * kb // 16)  # idx cols are 16-wrapped
    GCH = c["GCH"]
    DCHUNKS = c["DCHUNKS"]
    tail = nloc - 128 * (nb - 1)
    groups = [list(range(P))]

    nc = bacc.Bacc(
        "TRN2",
        target_bir_lowering=False,
        debug=False,
        num_devices=P,
        dynamic_dma_scratch_size=32768,
    )

    # I/O
    xt_d = nc.dram_tensor("xt", [FIN, nlocp], F32, kind="ExternalInput")
    w1_d = nc.dram_tensor("w1", [FIN, F1], F32, kind="ExternalInput")
    w2_d = nc.dram_tensor("w2", [F1, F2], F32, kind="ExternalInput")
    asrc1r_d = nc.dram_tensor("asrc1r", [128, F1], F32, kind="ExternalInput")
    adst1r_d = nc.dram_tensor("adst1r", [128, F1], F32, kind="ExternalInput")
    asrc2r_d = nc.dram_tensor("asrc2r", [128, F2], F32, kind="ExternalInput")
    adst2r_d = nc.dram_tensor("adst2r", [128, F2], F32, kind="ExternalInput")
    b1t_d = nc.dram_tensor("b1t", [128, 1], F32, kind="ExternalInput")
    b2t_d = nc.dram_tensor("b2t", [128, 1], F32, kind="ExternalInput")
    pads_d = nc.dram_tensor("pads", [2, 128], BF16, kind="ExternalInput")
    elli_d = nc.dram_tensor("elli", [128, ICOFF[-1]], I16, kind="ExternalInput")
    unp_d = nc.dram_tensor("unp", [128, nlocp // 16], I16, kind="ExternalInput")
    eye_d = nc.dram_tensor("eye", [128, 128], F32, kind="ExternalInput")
    out_d = nc.dram_tensor("out", [nloc, N], BF16, kind="ExternalOutput")

    # internal DRAM
    haug1 = nc.dram_tensor("haug1", [NR, 128], BF16, addr_space="Shared")
    haug2 = nc.dram_tensor("haug2", [NR, 128], BF16, addr_space="Shared")
    ccin1 = nc.dram_tensor("ccin1", [nloc, 128], BF16)
    ccout1 = nc.dram_tensor("ccout1", [N, 128], BF16, addr_space="Shared")
    ccin2 = nc.dram_tensor("ccin2", [nloc, 128], BF16)
    ccout2 = nc.dram_tensor("ccout2", [N, 128], BF16, addr_space="Shared")
    adt1 = nc.dram_tensor("adt1", [nb, 128], F32)
    adt2 = nc.dram_tensor("adt2", [nb, 128], F32)
    zperm = nc.dram_tensor("zperm", [nlocp, 64], F32)
    ztin = nc.dram_tensor("ztin", [F2, nloc], F32)
    ztcc = nc.dram_tensor("ztcc", [P * F2, nloc], F32, addr_space="Shared")

    with tile.TileContext(nc) as tc:
      with tc.tile_pool(name="persist", bufs=1) as ppool:
        zt_own = ppool.tile([F2, nlocp], F32R)
        def _pipeline():
            with (
                tc.tile_pool(name="const", bufs=1) as cpool,
                tc.tile_pool(name="strips", bufs=1) as spool,
                tc.tile_pool(name="gat_small", bufs=3) as gpool,
                tc.tile_pool(name="gather", bufs=2) as tpool,
                tc.tile_pool(name="psum_small", bufs=2, space="PSUM") as pps,
                tc.tile_pool(name="psum_agg", bufs=3, space="PSUM") as pagg,
                tc.tile_pool(name="psum_bc", bufs=2, space="PSUM") as pbc,
            ):
                # ---- constant loads -------------------------------------------------
                xt_sb = cpool.tile([FIN, nlocp], F32)
                nc.sync.dma_start(out=xt_sb[:, :], in_=xt_d[:, :])
                w1_sb = cpool.tile([FIN, F1], F32)
                nc.sync.dma_start(out=w1_sb[:, :], in_=w1_d[:, :])
                w2_sb = cpool.tile([F1, F2], F32)
                nc.sync.dma_start(out=w2_sb[:, :], in_=w2_d[:, :])
                asrc1_sb = cpool.tile([128, F1], F32)
                nc.sync.dma_start(out=asrc1_sb[:, :], in_=asrc1r_d[:, :])
                adst1_sb = cpool.tile([128, F1], F32)
                nc.sync.dma_start(out=adst1_sb[:, :], in_=adst1r_d[:, :])
                asrc2_sb = cpool.tile([128, F2], F32)
                nc.sync.dma_start(out=asrc2_sb[:, :], in_=asrc2r_d[:, :])
                adst2_sb = cpool.tile([128, F2], F32)
                nc.sync.dma_start(out=adst2_sb[:, :], in_=adst2r_d[:, :])
                b1t_sb = cpool.tile([128, 1], F32)
                nc.sync.dma_start(out=b1t_sb[:, :], in_=b1t_d[:, :])
                b2t_sb = cpool.tile([128, 1], F32)
                nc.sync.dma_start(out=b2t_sb[:, :], in_=b2t_d[:, :])
                elli_sb = cpool.tile([128, ICOFF[-1]], I16)
                nc.sync.dma_start(out=elli_sb[:, :], in_=elli_d[:, :])
                unp_sb = cpool.tile([128, nlocp // 16], I16)
                nc.sync.dma_start(out=unp_sb[:, :], in_=unp_d[:, :])
                eye_sb = cpool.tile([128, 128], F32)
                nc.sync.dma_start(out=eye_sb[:, :], in_=eye_d[:, :])
                ones1_sb = cpool.tile([1, 128], F32)
                nc.vector.memset(ones1_sb[:, :], 1.0)
                onesb_sb = cpool.tile([1, 64], BF16)
                nc.vector.memset(onesb_sb[:, :], 1.0)

                # selection matrix M: [128, K, 128] bf16, zeroed once; the
                # non-zero pattern (G strided diagonals) is identical for
                # every block, so reused buffers never need re-zeroing.
                m_sbs = []
                m_lastk = [KMAX, KMAX]
                for _mi in range(2):
                    m_i = cpool.tile([128, KMAX, 128], BF16, name=f"m{_mi}", tag=f"m{_mi}")
                    nc.vector.memset(m_i[:, :, :], 0.0)
                    m_sbs.append(m_i)

                strip = spool.tile([128, nb, 128], BF16)
                nc.vector.memset(strip[:, :, :], 0.0)
                zstrip = spool.tile([128, nb, 64], F32)
                nc.vector.memset(zstrip[:, :, :], 0.0)
                asv = spool.tile([128, nb], F32)
                adv1 = spool.tile([128, nb], F32)
                adv2 = spool.tile([128, nb], F32)
                adrep1 = spool.tile([128, nlocp], F32)
                adrep2 = spool.tile([128, nlocp], F32)

                # ---- helpers --------------------------------------------------------
                def hi_lo_to_strip(col0):
                    """Store asv [128, nb] f32 into strip cols col0 (bf16 hi)
                    and col0+1 (bf16 lo = asv - f32(hi))."""
                    hi_bf = gpool.tile([128, nb], BF16, tag="hibf")
                    nc.vector.tensor_copy(hi_bf[:, :], asv[:, :])
                    nc.vector.tensor_copy(strip[:, :, col0], hi_bf[:, :])
                    hi_f = gpool.tile([128, nb], F32, tag="hif")
                    nc.vector.tensor_copy(hi_f[:, :], hi_bf[:, :])
                    nc.vector.tensor_sub(hi_f[:, :], asv[:, :], hi_f[:, :])
                    nc.vector.tensor_copy(strip[:, :, col0 + 1], hi_f[:, :])

                def adrep_roundtrip(adv, adt_dram, adrep):
                    """adv [128, nb] (val for dst 128*b+p) -> adrep [128, nlocp]
                    (row-replicated) via DRAM roundtrip + ones-matmul broadcast."""
                    nc.sync.dma_start(out=adt_dram.ap().rearrange("b p -> p b"), in_=adv[:, :])
                    adrow = gpool.tile([1, nlocp], F32, tag="adrow")
                    nc.sync.dma_start(
                        out=adrow[:, :], in_=adt_dram.ap().flatten().unsqueeze(0)
                    )
                    for j0 in range(0, nlocp, 512):
                        w = min(512, nlocp - j0)
                        pt = pps.tile([128, 512], F32, tag="ps", name="ps")
                        nc.tensor.matmul(
                            pt[:, :w], ones1_sb[:, :], adrow[:, j0 : j0 + w],
                            start=True, stop=True,
                        )
                        nc.vector.tensor_copy(adrep[:, j0 : j0 + w], pt[:, :w])

                def strip_out(ccin):
                    if nb > 1:
                        nc.sync.dma_start(
                            out=ccin.ap()[0 : 128 * (nb - 1), :].rearrange(
                                "(b p) f -> p b f", p=128
                            ),
                            in_=strip[:, 0 : nb - 1, :],
                        )
                    nc.sync.dma_start(
                        out=ccin.ap()[128 * (nb - 1) : nloc, :],
                        in_=strip[0:tail, nb - 1, :],
                    )

                def allgather(ccin, ccout, haug, pad_idx):
                    # gather straight into haug[0:N] (no staging copy)
                    if sim_cc:
                        # TimelineSim proxy: approximate the AllGather's local
                        # receive traffic with P DRAM->DRAM copies.
                        for r in range(P):
                            nc.sync.dma_start(
                                out=haug.ap()[r * nloc : (r + 1) * nloc, :],
                                in_=ccin.ap()[:, :],
                            )
                    else:
                        nc.gpsimd.collective_compute(
                            "AllGather",
                            ALU.bypass,
                            replica_groups=groups,
                            ins=[ccin.ap().opt()],
                            outs=[haug.ap()[0:N, :].opt()],
                        )
                    nc.sync.dma_start(
                        out=haug.ap()[N : N + 1, :], in_=pads_d.ap()[pad_idx : pad_idx + 1, :]
                    )

                stopped = False

                def _dummy_out():
                    dz = gpool.tile([128, 512], BF16, tag="dz")
                    nc.vector.memset(dz[:, :], 0.0)
                    nc.sync.dma_start(out=out_d.ap()[0:128, 0:512], in_=dz[:, :])

                # ---- phase B: layer-1 linear on own nodes --------------------------
                scr = gpool.tile([128, F1], F32, tag="scr")
                for b in range(nb):
                    ph = pps.tile([128, 512], F32, tag="ps", name="ps")[:, 0:F1]
                    nc.tensor.matmul(
                        ph[:, :], xt_sb[:, 128 * b : 128 * (b + 1)], w1_sb[:, :],
                        start=True, stop=True,
                    )
                    nc.vector.tensor_copy(strip[:, b, 0:F1], ph[:, :])
                    nc.vector.memset(strip[:, b, 64:65], 1.0)
                    nc.vector.tensor_mul(scr[:, :], ph[:, :], asrc1_sb[:, :])
                    nc.vector.reduce_sum(
                        asv[:, b : b + 1], scr[:, :], axis=mybir.AxisListType.X
                    )
                    nc.vector.tensor_mul(scr[:, :], ph[:, :], adst1_sb[:, :])
                    nc.vector.reduce_sum(
                        adv1[:, b : b + 1], scr[:, :], axis=mybir.AxisListType.X
                    )
                hi_lo_to_strip(F1)
                if stop_after == "B0":
                    _dummy_out(); stopped = True
                if not stopped:
                    strip_out(ccin1)
                    allgather(ccin1, ccout1, haug1, 0)
                    adrep_roundtrip(adv1, adt1, adrep1)
                if stop_after == "B" and not stopped:
                    _dummy_out(); stopped = True

                # ---- edge layer ----------------------------------------------------
                EDGE_LVL = DEBUG_EDGE_LVL  # 3 = full edge phase (debug knob)

                def edge_layer(haug, adrep, fin, bt_sb, out_block, order):
                    """Aggregate one GAT layer for all own blocks.

                    haug rows (bf16): [h (fin) | 1 | as_hi | as_lo | junk]; for
                    each block produces zT = relu(aggT*rec + b) [fin, 128] and
                    calls out_block(b, zT).
                    """
                    scol = 64  # ones column -> denominator row of aggT
                    acol = fin  # as_hi / as_lo columns
                    rNc = 65  # matmul lhsT width: h cols .. ones col
                    for bi, b in enumerate(order):
                        Kb = KS[b]
                        T_p, adv, bnd = bands(Kb)
                        nS = Kb // T_p
                        T = tpool.tile([128, KMAX, 128], BF16, tag="T")
                        for q in range(128 * Kb // GCH):
                            nc.gpsimd.dma_gather(
                                T[:, q * (GCH // 128) : (q + 1) * (GCH // 128), :],
                                haug.ap()[:, :],
                                elli_sb[
                                    :,
                                    ICOFF[b] + q * (GCH // 16) : ICOFF[b] + (q + 1) * (GCH // 16),
                                ],
                                GCH,
                                GCH,
                                128,
                            )
                        if EDGE_LVL < 1:
                            continue
                        adT = gpool.tile([128, KMAX], F32, tag="adT")
                        for r, p0, p1, cc in bnd:
                            nc.vector.tensor_copy(
                                adT[p0:p1, r:Kb:T_p],
                                adrep[
                                    p0:p1,
                                    128 * b + cc : 128 * b + cc + adv * (nS - 1) + 1 : adv,
                                ],
                            )
                        e = gpool.tile([128, KMAX], F32, tag="e")
                        nc.vector.tensor_add(
                            e[:, 0:Kb], T[:, 0:Kb, acol], T[:, 0:Kb, acol + 1]
                        )
                        nc.vector.tensor_add(e[:, 0:Kb], e[:, 0:Kb], adT[:, 0:Kb])
                        # leaky_relu(e) = max(e, NEG*e), then exp on ACT
                        nc.vector.scalar_tensor_tensor(
                            e[:, 0:Kb], e[:, 0:Kb], NEG, e[:, 0:Kb], ALU.mult, ALU.max
                        )
                        ew = gpool.tile([128, KMAX], BF16, tag="ew")
                        nc.scalar.activation(ew[:, 0:Kb], e[:, 0:Kb], AF.Exp)
                        # scatter w into the fixed M pattern for width Kb;
                        # a buffer last used with a larger K keeps stale
                        # non-zeros outside the new pattern -> re-zero it
                        m_sb = m_sbs[bi % 2]
                        if Kb != m_lastk[bi % 2]:
                            # a K change moves every diagonal: clear the whole
                            # region either pattern touches
                            nc.vector.memset(
                                m_sb[:, 0 : max(Kb, m_lastk[bi % 2]), :], 0.0
                            )
                        m_lastk[bi % 2] = Kb
                        mv = m_sb[:, :, :].rearrange("p a b -> p (a b)")
                        cstr = 128 * T_p + adv
                        for r, p0, p1, cc in bnd:
                            nc.vector.tensor_copy(
                                mv[
                                    p0:p1,
                                    128 * r + cc : 128 * r + cc + (nS - 1) * cstr + 1 : cstr,
                                ],
                                ew[p0:p1, r:Kb:T_p],
                            )
                        if EDGE_LVL < 2:
                            continue
                        agg = pagg.tile([128, 128], F32, tag="agg")
                        for t in range(Kb):
                            nc.tensor.matmul(
                                agg[0:rNc, :],
                                T[:, t, 0:rNc],
                                m_sb[:, t, :],
                                start=(t == 0),
                                stop=(t == Kb - 1),
                            )
                        if EDGE_LVL < 3:
                            continue
                        rec = gpool.tile([1, 128], F32, tag="rec")
                        nc.vector.reciprocal(rec[:, :], agg[scol : scol + 1, :])
                        recb = gpool.tile([1, 128], BF16, tag="recb")
                        nc.vector.tensor_copy(recb[:, :], rec[:, :])
                        bc = pbc.tile([128, 128], F32, tag="bc")
                        nc.tensor.matmul(
                            bc[0:fin, :], onesb_sb[:, 0:fin], recb[:, :],
                            start=True, stop=True,
                        )
                        # DVE can't read two PSUM operands in one op: stage
                        # the broadcast in SBUF first.
                        bcs = gpool.tile([128, 128], F32, tag="bcs")
                        nc.vector.tensor_copy(bcs[0:fin, :], bc[0:fin, :])
                        z = gpool.tile([128, 128], F32, tag="z")
                        nc.vector.tensor_mul(z[0:fin, :], agg[0:fin, :], bcs[0:fin, :])
                        nc.vector.tensor_scalar(
                            z[0:fin, :], z[0:fin, :], bt_sb[0:fin, 0:1], 0.0,
                            ALU.add, ALU.max,
                        )
                        out_block(b, z)

                # ---- layer-1 consumer: h2 = z1 @ W2, rebuild strip -----------------
                def l1_out(b, z):
                    ph2 = pps.tile([128, 512], F32, tag="ps", name="ps")[:, 0:F2]
                    nc.tensor.matmul(
                        ph2[:, :], z[0:F1, :], w2_sb[:, :], start=True, stop=True
                    )
                    nc.vector.tensor_copy(strip[:, b, 0:F2], ph2[:, :])
                    nc.vector.memset(strip[:, b, 64:65], 1.0)
                    scr2 = gpool.tile([128, F2], F32, tag="scr2")
                    nc.vector.tensor_mul(scr2[:, :], ph2[:, :], asrc2_sb[:, :])
                    nc.vector.reduce_sum(
                        asv[:, b : b + 1], scr2[:, :], axis=mybir.AxisListType.X
                    )
                    nc.vector.tensor_mul(scr2[:, :], ph2[:, :], adst2_sb[:, :])
                    nc.vector.reduce_sum(
                        adv2[:, b : b + 1], scr2[:, :], axis=mybir.AxisListType.X
                    )

                if not stopped:
                    edge_layer(haug1, adrep1, F1, b1t_sb, l1_out, list(range(nb)))
                    hi_lo_to_strip(F2)
                if stop_after == "C" and not stopped:
                    _dummy_out(); stopped = True
                if not stopped:
                    strip_out(ccin2)
                    allgather(ccin2, ccout2, haug2, 1)
                    adrep_roundtrip(adv2, adt2, adrep2)

                # ---- layer-2 consumer: z rows (dst-major) into zstrip --------------
                def l2_out(b, z):
                    zp = pps.tile([128, 512], F32, tag="ps", name="ps")[:, 0:128]
                    nc.tensor.transpose(
                        zp[:, 0:F2], z[0:F2, :], eye_sb[0:F2, 0:F2]
                    )
                    nc.vector.tensor_copy(zstrip[:, b, 0:F2], zp[:, 0:F2])

                if not stopped:
                    # reverse order: layer 1 ends on the small-K tail, layer 2
                    # starts there -> only one K transition per layer
                    edge_layer(haug2, adrep2, F2, b2t_sb, l2_out, list(range(nb))[::-1])
                if stop_after == "D" and not stopped:
                    _dummy_out(); stopped = True

                if not stopped:
                    # un-permute: zstrip (sorted order) -> DRAM -> gather rows
                    # back in original order -> transpose into zt_own
                    nc.sync.dma_start(
                        out=zperm.ap().rearrange("(b p) f -> p b f", p=128),
                        in_=zstrip[:, :, :],
                    )
                    zrows = spool.tile([128, nb, 64], F32)
                    UGC = 512  # keep single-gather descriptor count small
                    for q in range(nlocp // UGC):
                        nc.gpsimd.dma_gather(
                            zrows[:, q * (UGC // 128) : (q + 1) * (UGC // 128), :],
                            zperm.ap()[:, :],
                            unp_sb[:, q * (UGC // 16) : (q + 1) * (UGC // 16)],
                            UGC,
                            UGC,
                            64,
                        )
                    for b in range(nb):
                        zp2 = pps.tile([128, 512], F32, tag="ps", name="ps")[:, 0:128]
                        nc.tensor.transpose(
                            zp2[0:F2, :], zrows[:, b, 0:F2], eye_sb[:, :]
                        )
                        nc.vector.tensor_copy(
                            zt_own[:, 128 * b : 128 * (b + 1)], zp2[0:F2, :]
                        )
                    # share z (transposed) with all cores
                    nc.sync.dma_start(out=ztin.ap()[:, :], in_=zt_own[:, 0:nloc].bitcast(F32))
                    if sim_cc:
                        for r in range(P):
                            nc.sync.dma_start(
                                out=ztcc.ap()[r * F2 : (r + 1) * F2, :],
                                in_=ztin.ap()[:, :],
                            )
                    else:
                        nc.gpsimd.collective_compute(
                            "AllGather",
                            ALU.bypass,
                            replica_groups=groups,
                            ins=[ztin.ap().opt()],
                            outs=[ztcc.ap().opt()],
                        )
                else:
                    nc.vector.memset(zt_own[:, :].bitcast(F32), 0.0)
                    nc.sync.dma_start(out=ztcc.ap()[0:F2, :], in_=zt_own[:, 0:nloc].bitcast(F32))

            # ---- decoder (separate pool scope so GAT SBUF is reusable) ------------
            with (
                tc.tile_pool(name="dec", bufs=1) as dpool,
                tc.tile_pool(name="dec_rows", bufs=2) as rpool,
                tc.tile_pool(name="psum_dec", bufs=2, space="PSUM") as pdec,
            ):
                if stopped:
                    P_eff = 0
                    nb_eff = 0
                else:
                    P_eff = P
                    nb_eff = nb
                ztf = dpool.tile([F2, N], F32)
                for r in range(P_eff):
                    nc.sync.dma_start(
                        out=ztf[:, r * nloc : (r + 1) * nloc],
                        in_=ztcc.ap()[r * F2 : (r + 1) * F2, :],
                    )
                for b in range(nb_eff):
                    rows = 128 if b < nb - 1 else tail
                    srow = rpool.tile([128, N], BF16, tag="srow")
                    j0 = 0
                    for W in DCHUNKS:
                        pd = pdec.tile([128, c["DCW"]], F32, tag="pd")
                        for s0 in range(0, W, 512):
                            w = min(512, W - s0)
                            nc.tensor.matmul(
                                pd[:, s0 : s0 + w],
                                zt_own[:, 128 * b : 128 * (b + 1)],
                                ztf[:, j0 + s0 : j0 + s0 + w].bitcast(F32R),
                                start=True,
                                stop=True,
                            )
                        nc.scalar.activation(
                            srow[:, j0 : j0 + W], pd[:, 0:W], AF.Sigmoid
                        )
                        j0 += W
                    nc.sync.dma_start(
                        out=out_d.ap()[128 * b : 128 * b + rows, :], in_=srow[0:rows, :]
                    )

        for _rep in range(repeat):
            _pipeline()
            if stop_after is not None and repeat > 1:
                tc.strict_bb_all_engine_barrier()

    nc.compile()
    return nc


# --------------------------------------------------------------------------
# entry point
# --------------------------------------------------------------------------

_CACHE = {}
TRACE = False
LAST_RESULT = None


def kernel(x, edge_index, W1, a_src1, a_dst1, b1, W2, a_src2, a_dst2, b2, **_):
    base = dict(FULL_CFG)
    # ELL width: 64 covers the reference graph (max in-degree 55); fall back
    # to 128 for denser graphs.
    ei = np.asarray(edge_index)
    deg = np.bincount(
        np.concatenate([ei[1].astype(np.int64), np.arange(base["N"])]),
        minlength=base["N"],
    )
    if deg.max() > 64:
        base["K"] = 128
    cfg = derive(base)
    maps = make_inputs(
        x, edge_index, W1, a_src1, a_dst1, b1, W2, a_src2, a_dst2, b2, cfg
    )
    key = ("full", cfg["KS"])  # program structure follows the K schedule
    if key not in _CACHE:
        _CACHE[key] = build_program(cfg)
    nc = _CACHE[key]
    global LAST_RESULT
    res = run_bass_kernel_spmd(nc, maps, list(range(cfg["P"])), trace=TRACE)
    LAST_RESULT = res
    out = np.concatenate(
        [np.asarray(res.results[i]["out"]) for i in range(cfg["P"])], axis=0
    )
    return out.astype(np.float32)
